# revision 5
# baseline (speedup 1.0000x reference)
"""Causal self-attention (B=4, T=2048, D=1024, H=16) on 8 Trainium2 cores. v2.

Sharding: tensor-parallel over heads — 2 heads per core. Each core computes
its QKV shard, causal attention for its heads, and a partial output
projection; the host sums the 8 partials.

v2 changes vs baseline:
  - all matmul inputs bf16 (x/w converted on host; q/k/v/P/attn bf16 on chip)
  - V projected directly into [tok, feat] layout (lhsT=x tile, rhs=w_v), so
    the separate V-transpose phase is gone
  - causal diagonal handled by a multiplicative 0/1 bf16 triangle mask on P
    after exp (replaces fp32 additive mask + masked exp): DVE work is a
    [128,128] bf16 4x-mode multiply per diagonal block
  - diagonal score blocks packed two-per-PSUM-pair-tile so pss pool covers
    both sub-diagonal pairs and diagonal blocks
  - out projection PSUM evacuation alternates ACT/DVE (Pool has no PSUM port)
  - output written bf16 (halves out DMA); host sums partials in fp32
"""

import os
import sys

sys.path.insert(0, "/opt/trn_rl_repo")

import numpy as np
import ml_dtypes
from contextlib import ExitStack

import concourse.bass as bass
import concourse.mybir as mybir
import concourse.tile as tile
from concourse import bacc
from concourse.bass_utils import run_bass_kernel_spmd

B, T, D, H, HD = 4, 2048, 1024, 16, 64
NCORES = 8
HPC = H // NCORES          # heads per core = 2
DC = HPC * HD              # per-core feature width = 128
TOK = B * T                # 8192
TB = T // 128              # tok tiles per batch = 16
F32 = mybir.dt.float32
BF16 = mybir.dt.bfloat16
EXP = mybir.ActivationFunctionType.Exp
SCALE = 1.0 / 8.0          # 1/sqrt(HD)
BF = ml_dtypes.bfloat16

LAST_RESULTS = None
PHASES = os.environ.get("K_PHASES", "123")
def _env_i(name, dflt):
    return int(os.environ.get(name, str(dflt)))
ATTNT_BUFS = _env_i("K_ATTNT_BUFS", 2)
XTS_BUFS = _env_i("K_XTS_BUFS", 3)
P_BUFS = _env_i("K_P_BUFS", 24)       # [128,1024] bf16 pair tiles
PD_BUFS = _env_i("K_PD_BUFS", 10)     # [128,2,512] bf16 diagonal tiles
TRI_EV = os.environ.get("K_TRI_EV", "p")   # p=Pool, v=DVE
S_BUFS = _env_i("K_S_BUFS", 2)        # [128,1024] fp32 psum pairs (2 banks ea)
AV_BUFS = _env_i("K_AV_BUFS", 2)
MM_BUFS = _env_i("K_MM_BUFS", 2)      # [128,512] fp32 psum (qkv/v/outproj)
QKV_EV = os.environ.get("K_QKV_EV", "v")   # engine for qk psum->sbuf copies
OB_EV = os.environ.get("K_OB_EV", "v")     # s=ACT, v=DVE, a=alternate
XDMA = _env_i("K_XDMA", 2)            # dma splits per x chunk


def _attention_kernel(tc, out, xT, wqkvT, woutT, trid, vonesd, tick):
    nc = tc.nc
    with ExitStack() as ctx:
        const = ctx.enter_context(tc.tile_pool(name="const", bufs=1))
        sb = ctx.enter_context(tc.tile_pool(name="sb", bufs=2))
        sb1 = ctx.enter_context(tc.tile_pool(name="sb1", bufs=ATTNT_BUFS))
        sbx = ctx.enter_context(tc.tile_pool(name="sbx", bufs=XTS_BUFS))
        sbp = ctx.enter_context(tc.tile_pool(name="sbp", bufs=P_BUFS))
        sbpd = ctx.enter_context(tc.tile_pool(name="sbpd", bufs=PD_BUFS))
        pss = ctx.enter_context(tc.tile_pool(name="pss", bufs=S_BUFS, space="PSUM"))
        psav = ctx.enter_context(tc.tile_pool(name="psav", bufs=AV_BUFS, space="PSUM"))
        pmm = ctx.enter_context(tc.tile_pool(name="pmm", bufs=MM_BUFS, space="PSUM"))

        # ---- constants ----
        w_sb = const.tile([128, 8, 3 * DC], BF16, tag="wqkv")
        nc.sync.dma_start(out=w_sb, in_=wqkvT.rearrange("(dt p) f -> p dt f", p=128))
        wo_sb = const.tile([128, D], BF16, tag="wout")
        nc.sync.dma_start(out=wo_sb, in_=woutT)
        tri2 = const.tile([128, 2, 128], BF16, tag="tri2")
        nc.sync.dma_start(out=tri2, in_=trid.rearrange("p (h c) -> p h c", h=2))
        nc.sync.dma_start(out=tick, in_=trid[:, 0:tick.shape[1]])
        # persistent V tiles [128 ktok, TB, 2 heads, 64 data + 64 ones],
        # ping-ponged across batches to avoid WAR stalls on batch b+1's
        # phase-1 V writes vs batch b's AV reads; ones written once here
        vvs = []
        for i in range(2):
            vv = const.tile([128, TB, 2, 128], BF16, tag=f"vv{i}")
            nc.sync.dma_start(out=vv[:, :, :, 64:128], in_=vonesd)
            vvs.append(vv)

        xTr = xT.rearrange("(dt p) tok -> p dt tok", p=128)

        pools = (sb, sb1, sbx, sbp, sbpd, pss, psav, pmm)

        def body():
            _kernel_body(tc, out, xTr, w_sb, wo_sb, tri2, vvs, pools)

        nloop = int(os.environ.get("K_LOOP", "1"))
        if nloop > 1:
            with tc.For_i(0, nloop, 1):
                body()
        else:
            body()


def _kernel_body(tc, out, xTr, w_sb, wo_sb, tri2, vvs, pools):
    (sb, sb1, sbx, sbp, sbpd, pss, psav, pmm) = pools
    nc = tc.nc

    qkTs = {}

    def phase1_chunk_units(b, ci):
        """QKV projection for one 512-token chunk of batch b, as a list of
        emission thunks so they can be woven between score units."""
        def u_load():
            vv = vvs[b % 2]
            if ci == 0:
                qTt = sb.tile([128, T], BF16, tag="qT")
                kTt = sb.tile([128, T], BF16, tag="kT")
                qkTs[b] = (qTt, kTt)
            tok0 = b * T + ci * 512
            xts = sbx.tile([128, 8, 512], BF16, tag="xts")
            dsp = 8 // XDMA
            for di in range(XDMA):
                nc.sync.dma_start(
                    out=xts[:, di * dsp:(di + 1) * dsp, :],
                    in_=xTr[:, di * dsp:(di + 1) * dsp, tok0:tok0 + 512])
            qkTs["xts"] = xts

        def u_qk(ft):
            def f():
                xts = qkTs["xts"]
                qkvp = pmm.tile([128, 512], F32, tag="mm")
                for dt in range(8):
                    nc.tensor.matmul(
                        qkvp,
                        w_sb[:, dt, ft * DC:(ft + 1) * DC],
                        xts[:, dt, :],
                        start=(dt == 0), stop=(dt == 7),
                    )
                dst = qkTs[b][ft][:, ci * 512:(ci + 1) * 512]
                if QKV_EV == "v":
                    nc.vector.tensor_copy(dst, qkvp)
                else:
                    nc.scalar.copy(dst, qkvp)
            return f

        def u_v():
            # V directly in [tok, feat] layout: lhsT = x tile, rhs = w_v
            vv = vvs[b % 2]
            xts = qkTs["xts"]
            vp = pmm.tile([128, 4, 128], F32, tag="mm")
            for tt in range(4):
                for dt in range(8):
                    nc.tensor.matmul(
                        vp[:, tt, :],
                        xts[:, dt, tt * 128:(tt + 1) * 128],
                        w_sb[:, dt, 2 * DC:3 * DC],
                        start=(dt == 0), stop=(dt == 7),
                    )
            # cols 0:64 = head0 feats, 64:128 = head1 -> vv data slots
            nc.vector.tensor_copy(
                vv[:, ci * 4:(ci + 1) * 4, :, 0:64],
                vp.rearrange("p t (h c) -> p t h c", h=2))

        return [u_load, u_qk(0), u_qk(1), u_v]

    def phase1_chunk(b, ci):
        for u in phase1_chunk_units(b, ci):
            u()

    if "1" not in PHASES:
        return
    for ci in range(4):
        phase1_chunk(0, ci)

    for b in range(B):
        vv = vvs[b % 2]
        qT, kT = qkTs[b]
        attnT = sb1.tile([128, T], BF16, tag="attnT")
        if "2" not in PHASES:
            if b + 1 < B:
                for ci in range(4):
                    phase1_chunk(b + 1, ci)
            continue

        # ====== phase 2+3: attention per q-chunk, fused outproj ======
        ob_state = [0]

        def outproj_tt(tt):
            def f():
                ob = sb.tile([128, 1024], BF16, tag="ob")
                for fc in range(2):
                    op_ = pmm.tile([128, 512], F32, tag="mm")
                    nc.tensor.matmul(
                        op_,
                        attnT[:, tt * 128:(tt + 1) * 128],
                        wo_sb[:, fc * 512:(fc + 1) * 512],
                        start=True, stop=True,
                    )
                    use_act = (OB_EV == "s"
                               or (OB_EV == "a" and ob_state[0] == 0))
                    ob_state[0] ^= 1
                    if use_act:
                        nc.scalar.copy(ob[:, fc * 512:(fc + 1) * 512], op_)
                    else:
                        nc.vector.tensor_copy(ob[:, fc * 512:(fc + 1) * 512],
                                              op_)
                row0 = b * T + tt * 128
                nc.sync.dma_start(out=out[row0:row0 + 128, :], in_=ob)
            return f

        def outproj_units(qb):
            return [outproj_tt(tt) for tt in range(4 * qb, 4 * qb + 4)]

        def outproj(qb):
            if "3" not in PHASES:
                return
            for u in outproj_units(qb):
                u()

        for qb in range(4):                      # 512-wide q chunks
            nkt = 4 * (qb + 1)
            nsub = 4 * qb                        # sub-diagonal k-tiles
            qs = qb * 512
            avlists = ([], [])

            def u_scpair(h, kt):
                def f():
                    sp = pss.tile([128, 1024], F32, tag="s")
                    pt = sbp.tile([128, 1024], BF16, tag="p")
                    for j in range(2):
                        nc.tensor.matmul(
                            sp[:, j * 512:(j + 1) * 512],
                            kT[h * 64:(h + 1) * 64,
                               (kt + j) * 128:(kt + j + 1) * 128],
                            qT[h * 64:(h + 1) * 64, qs:qs + 512],
                            start=True, stop=True,
                        )
                    nc.scalar.activation(pt, sp, EXP, scale=SCALE)
                    avlists[h].append((kt, pt[:, 0:512], 0))
                    avlists[h].append((kt + 1, pt[:, 512:1024], 0))
                return f

            def u_scdiag(o):
                # both heads' diagonal block o share one psum pair tile, so
                # ONE exp + ONE triangle-mask instruction cover both heads;
                # only columns q >= 128*o live
                def f():
                    kt = nsub + o
                    off = 128 * o
                    spd = pss.tile([128, 2, 512], F32, tag="s")
                    for h in range(HPC):
                        nc.tensor.matmul(
                            spd[:, h, off:512],
                            kT[h * 64:(h + 1) * 64,
                               kt * 128:(kt + 1) * 128],
                            qT[h * 64:(h + 1) * 64, qs + off:qs + 512],
                            start=True, stop=True,
                        )
                    pt = sbpd.tile([128, 2, 512], BF16, tag="pd")
                    nc.scalar.activation(pt[:, :, off:512],
                                         spd[:, :, off:512],
                                         EXP, scale=SCALE)
                    # SBUF-only operands, so the otherwise-idle Pool engine
                    # can mask the triangle, keeping DVE off the exp->AV path
                    tri_eng = nc.gpsimd if TRI_EV == "p" else nc.vector
                    tri_eng.tensor_tensor(
                        out=pt[:, :, off:off + 128],
                        in0=pt[:, :, off:off + 128],
                        in1=tri2, op=mybir.AluOpType.mult,
                    )
                    for h in range(HPC):
                        avlists[h].append((kt, pt[:, h, off:512], off))
                return f

            sc_units = []
            for h in range(HPC):
                for kt in range(0, nsub, 2):
                    sc_units.append(u_scpair(h, kt))
            for o in range(4):
                sc_units.append(u_scdiag(o))

            # PE fillers woven between score units: previous chunk's out
            # projection + next batch's phase-1 chunk cover exp latency
            fillers = []
            if qb > 0 and "3" in PHASES:
                fillers += outproj_units(qb - 1)
            if b + 1 < B:
                fillers += phase1_chunk_units(b + 1, qb)

            # weave fillers evenly between score units (after every other
            # score unit); leftovers land before the AV chains
            fi = 0
            for i, u in enumerate(sc_units):
                u()
                if i % 2 == 1 and fi < len(fillers):
                    fillers[fi]()
                    fi += 1
            while fi < len(fillers):
                fillers[fi]()
                fi += 1

            for h in range(HPC):
                avp = psav.tile([128, 512], F32, tag="av")
                for kt, psl, off in avlists[h]:
                    nc.tensor.matmul(
                        avp[:, off:512],
                        vv[:, kt, h, :],
                        psl,
                        start=(kt == 0), stop=(kt == nkt - 1),
                    )
                rc = sb.tile([128, 512], F32, tag="recip")
                nc.vector.reciprocal(rc[0:64, :], avp[64:128, :])
                nc.vector.tensor_tensor(
                    out=attnT[h * 64:(h + 1) * 64, qs:qs + 512],
                    in0=avp[0:64, :], in1=rc[0:64, :],
                    op=mybir.AluOpType.mult,
                )
        outproj(3)


def build_module():
    nc = bacc.Bacc("TRN2", target_bir_lowering=False, debug=False,
                   num_devices=NCORES)
    xT = nc.declare_dram_parameter("xT", [D, TOK], BF16, isOutput=False)
    wqkvT = nc.declare_dram_parameter("wqkvT", [D, 3 * DC], BF16, isOutput=False)
    woutT = nc.declare_dram_parameter("woutT", [DC, D], BF16, isOutput=False)
    trid = nc.declare_dram_parameter("tri", [128, 256], BF16, isOutput=False)
    vonesd = nc.declare_dram_parameter("vones", [128, TB * 2 * 64], BF16, isOutput=False)
    out = nc.declare_dram_parameter("out", [TOK, D], BF16, isOutput=True)
    # tick's shape varies with K_LOOP so the two A/B perf modules lower to
    # DIFFERENT HLO: identical shapes collide in the jit/NEFF cache and the
    # loop module silently runs the 1x NEFF
    nloop = int(os.environ.get("K_LOOP", "1"))
    tick = nc.declare_dram_parameter(
        "tick", [128, 8 + 8 * min(nloop - 1, 1)], BF16, isOutput=True)
    with tile.TileContext(nc) as tc:
        _attention_kernel(
            tc, out[:], xT[:], wqkvT[:], woutT[:], trid[:],
            vonesd[:].rearrange("p (t h c) -> p t h c", c=64, h=2), tick[:])
    nc.compile()
    return nc


def shard_inputs(x, w_qkv, w_out):
    """Returns per-core input maps."""
    x_flat = np.asarray(x, np.float32).reshape(TOK, D)
    xT = np.ascontiguousarray(x_flat.T).astype(BF)   # [D, TOK]
    w_qkv = np.asarray(w_qkv, np.float32)
    w_out = np.asarray(w_out, np.float32)
    kp = np.arange(128)[:, None]
    jq = np.arange(128)[None, :]
    tri1 = (kp <= jq).astype(BF)                     # [128,128] lower-left 0/1
    trid = np.concatenate([tri1, tri1], axis=1)      # duplicated per head
    vones = np.ones((128, TB * 2 * 64), BF)
    in_maps = []
    for c in range(NCORES):
        r0 = c * DC
        wq = w_qkv[r0:r0 + DC]                   # Q rows for heads 2c, 2c+1
        wk = w_qkv[D + r0:D + r0 + DC]
        wv = w_qkv[2 * D + r0:2 * D + r0 + DC]
        wqkvT = np.ascontiguousarray(
            np.concatenate([wq, wk, wv], axis=0).T).astype(BF)   # [D, 3*DC]
        woutT = np.ascontiguousarray(w_out[:, r0:r0 + DC].T).astype(BF)
        in_maps.append({"xT": xT, "wqkvT": wqkvT, "woutT": woutT,
                        "tri": trid, "vones": vones})
    return in_maps


_NC_CACHE = None


def kernel(x, w_qkv, w_out):
    global _NC_CACHE, LAST_RESULTS
    if _NC_CACHE is None:
        _NC_CACHE = build_module()
    nc = _NC_CACHE
    in_maps = shard_inputs(x, w_qkv, w_out)
    os.environ["BASS_NEVER_TRACE"] = "1"
    res = run_bass_kernel_spmd(nc, in_maps, list(range(NCORES)), trace=False)
    LAST_RESULTS = res
    acc = np.zeros((TOK, D), dtype=np.float32)
    for r in res.results:
        acc += np.asarray(r["out"], dtype=np.float32)
    return acc.reshape(B, T, D)


# revision 8
# speedup vs baseline: 1.0950x; 1.0950x over previous
"""Causal self-attention (B=4, T=2048, D=1024, H=16) on 8 Trainium2 cores. v2.

Sharding: tensor-parallel over heads — 2 heads per core. Each core computes
its QKV shard, causal attention for its heads, and a partial output
projection; the host sums the 8 partials.

v2 changes vs baseline:
  - all matmul inputs bf16 (x/w converted on host; q/k/v/P/attn bf16 on chip)
  - V projected directly into [tok, feat] layout (lhsT=x tile, rhs=w_v), so
    the separate V-transpose phase is gone
  - causal diagonal handled by a multiplicative 0/1 bf16 triangle mask on P
    after exp (replaces fp32 additive mask + masked exp): DVE work is a
    [128,128] bf16 4x-mode multiply per diagonal block
  - diagonal score blocks packed two-per-PSUM-pair-tile so pss pool covers
    both sub-diagonal pairs and diagonal blocks
  - out projection PSUM evacuation alternates ACT/DVE (Pool has no PSUM port)
  - output written bf16 (halves out DMA); host sums partials in fp32
"""

import os
import sys

sys.path.insert(0, "/opt/trn_rl_repo")

import numpy as np
import ml_dtypes
from contextlib import ExitStack

import concourse.bass as bass
import concourse.mybir as mybir
import concourse.tile as tile
from concourse import bacc
from concourse.bass_utils import run_bass_kernel_spmd

B, T, D, H, HD = 4, 2048, 1024, 16, 64
NCORES = 8
HPC = H // NCORES          # heads per core = 2
DC = HPC * HD              # per-core feature width = 128
TOK = B * T                # 8192
TB = T // 128              # tok tiles per batch = 16
F32 = mybir.dt.float32
BF16 = mybir.dt.bfloat16
EXP = mybir.ActivationFunctionType.Exp
SCALE = 1.0 / 8.0          # 1/sqrt(HD)
BF = ml_dtypes.bfloat16

LAST_RESULTS = None
PHASES = os.environ.get("K_PHASES", "123")
def _env_i(name, dflt):
    return int(os.environ.get(name, str(dflt)))
ATTNT_BUFS = _env_i("K_ATTNT_BUFS", 2)
XTS_BUFS = _env_i("K_XTS_BUFS", 3)
P_BUFS = _env_i("K_P_BUFS", 24)       # [128,1024] bf16 pair tiles
PD_BUFS = _env_i("K_PD_BUFS", 10)     # [128,2,512] bf16 diagonal tiles
# Pool tri measured 527us vs DVE 420us on HW — q7 launch overhead sits on
# the critical exp->mask->AV path; keep DVE
TRI_EV = os.environ.get("K_TRI_EV", "v")   # p=Pool, v=DVE
S_BUFS = _env_i("K_S_BUFS", 2)        # [128,1024] fp32 psum pairs (2 banks ea)
AV_BUFS = _env_i("K_AV_BUFS", 2)
MM_BUFS = _env_i("K_MM_BUFS", 2)      # [128,512] fp32 psum (qkv/v/outproj)
QKV_EV = os.environ.get("K_QKV_EV", "v")   # engine for qk psum->sbuf copies
OB_EV = os.environ.get("K_OB_EV", "v")     # s=ACT, v=DVE, a=alternate
XDMA = _env_i("K_XDMA", 2)            # dma splits per x chunk


def _attention_kernel(tc, out, xT, wqkvT, woutT, trid, vonesd, tick):
    nc = tc.nc
    with ExitStack() as ctx:
        const = ctx.enter_context(tc.tile_pool(name="const", bufs=1))
        sb = ctx.enter_context(tc.tile_pool(name="sb", bufs=2))
        sb1 = ctx.enter_context(tc.tile_pool(name="sb1", bufs=ATTNT_BUFS))
        sbx = ctx.enter_context(tc.tile_pool(name="sbx", bufs=XTS_BUFS))
        sbp = ctx.enter_context(tc.tile_pool(name="sbp", bufs=P_BUFS))
        sbpd = ctx.enter_context(tc.tile_pool(name="sbpd", bufs=PD_BUFS))
        pss = ctx.enter_context(tc.tile_pool(name="pss", bufs=S_BUFS, space="PSUM"))
        psav = ctx.enter_context(tc.tile_pool(name="psav", bufs=AV_BUFS, space="PSUM"))
        pmm = ctx.enter_context(tc.tile_pool(name="pmm", bufs=MM_BUFS, space="PSUM"))

        # ---- constants ----
        # w_sb rides the SP queue ahead of the first x chunk; all other
        # constants go on the ACT hwdge queue so the first QKV matmul's
        # xts DMA isn't stuck behind them (single-shot startup latency)
        w_sb = const.tile([128, 8, 3 * DC], BF16, tag="wqkv")
        nc.sync.dma_start(out=w_sb, in_=wqkvT.rearrange("(dt p) f -> p dt f", p=128))
        wo_sb = const.tile([128, D], BF16, tag="wout")
        nc.scalar.dma_start(out=wo_sb, in_=woutT)
        tri2 = const.tile([128, 2, 128], BF16, tag="tri2")
        nc.scalar.dma_start(out=tri2, in_=trid.rearrange("p (h c) -> p h c", h=2))
        nc.scalar.dma_start(out=tick, in_=trid[:, 0:tick.shape[1]])
        # persistent V tiles [128 ktok, TB, 2 heads, 64 data + 64 ones],
        # ping-ponged across batches to avoid WAR stalls on batch b+1's
        # phase-1 V writes vs batch b's AV reads; ones written once here
        vvs = []
        for i in range(2):
            vv = const.tile([128, TB, 2, 128], BF16, tag=f"vv{i}")
            nc.scalar.dma_start(out=vv[:, :, :, 64:128], in_=vonesd)
            vvs.append(vv)

        xTr = xT.rearrange("(dt p) tok -> p dt tok", p=128)

        pools = (sb, sb1, sbx, sbp, sbpd, pss, psav, pmm)

        def body():
            _kernel_body(tc, out, xTr, w_sb, wo_sb, tri2, vvs, pools)

        nloop = int(os.environ.get("K_LOOP", "1"))
        if nloop > 1:
            with tc.For_i(0, nloop, 1):
                body()
        else:
            body()


def _kernel_body(tc, out, xTr, w_sb, wo_sb, tri2, vvs, pools):
    (sb, sb1, sbx, sbp, sbpd, pss, psav, pmm) = pools
    nc = tc.nc

    qkTs = {}

    def phase1_chunk_units(b, ci):
        """QKV projection for one 512-token chunk of batch b, as a list of
        emission thunks so they can be woven between score units."""
        def u_load():
            vv = vvs[b % 2]
            if ci == 0:
                qTt = sb.tile([128, T], BF16, tag="qT")
                kTt = sb.tile([128, T], BF16, tag="kT")
                qkTs[b] = (qTt, kTt)
            tok0 = b * T + ci * 512
            xts = sbx.tile([128, 8, 512], BF16, tag="xts")
            dsp = 8 // XDMA
            for di in range(XDMA):
                nc.sync.dma_start(
                    out=xts[:, di * dsp:(di + 1) * dsp, :],
                    in_=xTr[:, di * dsp:(di + 1) * dsp, tok0:tok0 + 512])
            qkTs["xts"] = xts

        def u_qk(ft):
            def f():
                xts = qkTs["xts"]
                qkvp = pmm.tile([128, 512], F32, tag="mm")
                for dt in range(8):
                    nc.tensor.matmul(
                        qkvp,
                        w_sb[:, dt, ft * DC:(ft + 1) * DC],
                        xts[:, dt, :],
                        start=(dt == 0), stop=(dt == 7),
                    )
                dst = qkTs[b][ft][:, ci * 512:(ci + 1) * 512]
                if QKV_EV == "v":
                    nc.vector.tensor_copy(dst, qkvp)
                else:
                    nc.scalar.copy(dst, qkvp)
            return f

        def u_v():
            # V directly in [tok, feat] layout: lhsT = x tile, rhs = w_v
            vv = vvs[b % 2]
            xts = qkTs["xts"]
            vp = pmm.tile([128, 4, 128], F32, tag="mm")
            for tt in range(4):
                for dt in range(8):
                    nc.tensor.matmul(
                        vp[:, tt, :],
                        xts[:, dt, tt * 128:(tt + 1) * 128],
                        w_sb[:, dt, 2 * DC:3 * DC],
                        start=(dt == 0), stop=(dt == 7),
                    )
            # cols 0:64 = head0 feats, 64:128 = head1 -> vv data slots
            nc.vector.tensor_copy(
                vv[:, ci * 4:(ci + 1) * 4, :, 0:64],
                vp.rearrange("p t (h c) -> p t h c", h=2))

        return [u_load, u_qk(0), u_qk(1), u_v]

    def phase1_chunk(b, ci):
        for u in phase1_chunk_units(b, ci):
            u()

    if "1" not in PHASES:
        return
    for ci in range(4):
        phase1_chunk(0, ci)

    for b in range(B):
        vv = vvs[b % 2]
        qT, kT = qkTs[b]
        attnT = sb1.tile([128, T], BF16, tag="attnT")
        if "2" not in PHASES:
            if b + 1 < B:
                for ci in range(4):
                    phase1_chunk(b + 1, ci)
            continue

        # ====== phase 2+3: attention per q-chunk, fused outproj ======
        ob_state = [0]

        def outproj_tt(tt):
            def f():
                ob = sb.tile([128, 1024], BF16, tag="ob")
                for fc in range(2):
                    op_ = pmm.tile([128, 512], F32, tag="mm")
                    nc.tensor.matmul(
                        op_,
                        attnT[:, tt * 128:(tt + 1) * 128],
                        wo_sb[:, fc * 512:(fc + 1) * 512],
                        start=True, stop=True,
                    )
                    use_act = (OB_EV == "s"
                               or (OB_EV == "a" and ob_state[0] == 0))
                    ob_state[0] ^= 1
                    if use_act:
                        nc.scalar.copy(ob[:, fc * 512:(fc + 1) * 512], op_)
                    else:
                        nc.vector.tensor_copy(ob[:, fc * 512:(fc + 1) * 512],
                                              op_)
                row0 = b * T + tt * 128
                nc.sync.dma_start(out=out[row0:row0 + 128, :], in_=ob)
            return f

        def outproj_units(qb):
            return [outproj_tt(tt) for tt in range(4 * qb, 4 * qb + 4)]

        def outproj(qb):
            if "3" not in PHASES:
                return
            for u in outproj_units(qb):
                u()

        for qb in range(4):                      # 512-wide q chunks
            nkt = 4 * (qb + 1)
            nsub = 4 * qb                        # sub-diagonal k-tiles
            qs = qb * 512
            avlists = ([], [])

            def u_scpair(h, kt):
                def f():
                    sp = pss.tile([128, 1024], F32, tag="s")
                    pt = sbp.tile([128, 1024], BF16, tag="p")
                    for j in range(2):
                        nc.tensor.matmul(
                            sp[:, j * 512:(j + 1) * 512],
                            kT[h * 64:(h + 1) * 64,
                               (kt + j) * 128:(kt + j + 1) * 128],
                            qT[h * 64:(h + 1) * 64, qs:qs + 512],
                            start=True, stop=True,
                        )
                    nc.scalar.activation(pt, sp, EXP, scale=SCALE)
                    avlists[h].append((kt, pt[:, 0:512], 0))
                    avlists[h].append((kt + 1, pt[:, 512:1024], 0))
                return f

            def u_scdiag(o):
                # both heads' diagonal block o share one psum pair tile, so
                # ONE exp + ONE triangle-mask instruction cover both heads;
                # only columns q >= 128*o live
                def f():
                    kt = nsub + o
                    off = 128 * o
                    spd = pss.tile([128, 2, 512], F32, tag="s")
                    for h in range(HPC):
                        nc.tensor.matmul(
                            spd[:, h, off:512],
                            kT[h * 64:(h + 1) * 64,
                               kt * 128:(kt + 1) * 128],
                            qT[h * 64:(h + 1) * 64, qs + off:qs + 512],
                            start=True, stop=True,
                        )
                    pt = sbpd.tile([128, 2, 512], BF16, tag="pd")
                    nc.scalar.activation(pt[:, :, off:512],
                                         spd[:, :, off:512],
                                         EXP, scale=SCALE)
                    # SBUF-only operands, so the otherwise-idle Pool engine
                    # can mask the triangle, keeping DVE off the exp->AV path
                    tri_eng = nc.gpsimd if TRI_EV == "p" else nc.vector
                    tri_eng.tensor_tensor(
                        out=pt[:, :, off:off + 128],
                        in0=pt[:, :, off:off + 128],
                        in1=tri2, op=mybir.AluOpType.mult,
                    )
                    for h in range(HPC):
                        avlists[h].append((kt, pt[:, h, off:512], off))
                return f

            sc_units = []
            for h in range(HPC):
                for kt in range(0, nsub, 2):
                    sc_units.append(u_scpair(h, kt))
            for o in range(4):
                sc_units.append(u_scdiag(o))

            # PE fillers woven between score units: previous chunk's out
            # projection + next batch's phase-1 chunk cover exp latency
            fillers = []
            if qb > 0 and "3" in PHASES:
                fillers += outproj_units(qb - 1)
            if b + 1 < B:
                fillers += phase1_chunk_units(b + 1, qb)

            # weave fillers evenly between score units (after every other
            # score unit); leftovers land before the AV chains
            fi = 0
            for i, u in enumerate(sc_units):
                u()
                if i % 2 == 1 and fi < len(fillers):
                    fillers[fi]()
                    fi += 1
            while fi < len(fillers):
                fillers[fi]()
                fi += 1

            for h in range(HPC):
                avp = psav.tile([128, 512], F32, tag="av")
                for kt, psl, off in avlists[h]:
                    nc.tensor.matmul(
                        avp[:, off:512],
                        vv[:, kt, h, :],
                        psl,
                        start=(kt == 0), stop=(kt == nkt - 1),
                    )
                rc = sb.tile([128, 512], F32, tag="recip")
                nc.vector.reciprocal(rc[0:64, :], avp[64:128, :])
                nc.vector.tensor_tensor(
                    out=attnT[h * 64:(h + 1) * 64, qs:qs + 512],
                    in0=avp[0:64, :], in1=rc[0:64, :],
                    op=mybir.AluOpType.mult,
                )
        outproj(3)


def build_module():
    nc = bacc.Bacc("TRN2", target_bir_lowering=False, debug=False,
                   num_devices=NCORES)
    xT = nc.declare_dram_parameter("xT", [D, TOK], BF16, isOutput=False)
    wqkvT = nc.declare_dram_parameter("wqkvT", [D, 3 * DC], BF16, isOutput=False)
    woutT = nc.declare_dram_parameter("woutT", [DC, D], BF16, isOutput=False)
    trid = nc.declare_dram_parameter("tri", [128, 256], BF16, isOutput=False)
    vonesd = nc.declare_dram_parameter("vones", [128, TB * 2 * 64], BF16, isOutput=False)
    out = nc.declare_dram_parameter("out", [TOK, D], BF16, isOutput=True)
    # tick's shape varies with K_LOOP so the two A/B perf modules lower to
    # DIFFERENT HLO: identical shapes collide in the jit/NEFF cache and the
    # loop module silently runs the 1x NEFF
    nloop = int(os.environ.get("K_LOOP", "1"))
    tick = nc.declare_dram_parameter(
        "tick", [128, 8 + 8 * min(nloop - 1, 1)], BF16, isOutput=True)
    with tile.TileContext(nc) as tc:
        _attention_kernel(
            tc, out[:], xT[:], wqkvT[:], woutT[:], trid[:],
            vonesd[:].rearrange("p (t h c) -> p t h c", c=64, h=2), tick[:])
    nc.compile()
    return nc


def shard_inputs(x, w_qkv, w_out):
    """Returns per-core input maps."""
    x_flat = np.asarray(x, np.float32).reshape(TOK, D)
    xT = np.ascontiguousarray(x_flat.T).astype(BF)   # [D, TOK]
    w_qkv = np.asarray(w_qkv, np.float32)
    w_out = np.asarray(w_out, np.float32)
    kp = np.arange(128)[:, None]
    jq = np.arange(128)[None, :]
    tri1 = (kp <= jq).astype(BF)                     # [128,128] lower-left 0/1
    trid = np.concatenate([tri1, tri1], axis=1)      # duplicated per head
    vones = np.ones((128, TB * 2 * 64), BF)
    in_maps = []
    for c in range(NCORES):
        r0 = c * DC
        wq = w_qkv[r0:r0 + DC]                   # Q rows for heads 2c, 2c+1
        wk = w_qkv[D + r0:D + r0 + DC]
        wv = w_qkv[2 * D + r0:2 * D + r0 + DC]
        wqkvT = np.ascontiguousarray(
            np.concatenate([wq, wk, wv], axis=0).T).astype(BF)   # [D, 3*DC]
        woutT = np.ascontiguousarray(w_out[:, r0:r0 + DC].T).astype(BF)
        in_maps.append({"xT": xT, "wqkvT": wqkvT, "woutT": woutT,
                        "tri": trid, "vones": vones})
    return in_maps


_NC_CACHE = None


def kernel(x, w_qkv, w_out):
    global _NC_CACHE, LAST_RESULTS
    if _NC_CACHE is None:
        _NC_CACHE = build_module()
    nc = _NC_CACHE
    in_maps = shard_inputs(x, w_qkv, w_out)
    os.environ["BASS_NEVER_TRACE"] = "1"
    res = run_bass_kernel_spmd(nc, in_maps, list(range(NCORES)), trace=False)
    LAST_RESULTS = res
    acc = np.zeros((TOK, D), dtype=np.float32)
    for r in res.results:
        acc += np.asarray(r["out"], dtype=np.float32)
    return acc.reshape(B, T, D)
